# revision 6
# baseline (speedup 1.0000x reference)
"""MiniMax-M2 sparse MoE block on 8 Trainium2 NeuronCores.

Strategy: expert-parallel. Core c owns expert c's weights. The host computes
the routing (top-2 of 8, sigmoid scores + bias for selection) in float64,
gathers each expert's tokens, and ships them pre-transposed; each core runs
the gated FFN (silu(x@w1.T) * (x@w3.T)) @ w2.T over its gathered tokens in
float32r (full-rate fp32 matmul mode) and also computes the router logits for
its 1/8 slice of tokens. The host scatter-adds the per-expert outputs.

Shapes (fixed by the problem): B=4, S=4096, H=2048, I=1024, E=8, top_k=2.

The intermediate dim I is processed in two halves so that one half's weights
(w1/w3 halves [H,512] + w2 half [512,H], fp32) stay resident in SBUF; each
half emits a partial Y (summed on the host).
"""
import math
import numpy as np

import concourse.bass as bass
import concourse.mybir as mybir
from concourse import bacc
import concourse.tile as tile
from concourse.bass_utils import run_bass_kernel_spmd

H = 2048
I = 1024
E = 8
TOP_K = 2
KT_H = H // 128          # 16 k-tiles over H
IH = I // 2              # 512: half of intermediate dim
ISUB = IH // 128         # 4 psum subtiles per half
KT_I = IH // 128         # 4 k-tiles over I-half
NTOK = 512               # token tile (matmul moving dim)
TSLICE = None            # tokens per core for the router (T/8), set at build
f32 = mybir.dt.float32
f32r = mybir.dt.float32r
ACT_FN = mybir.ActivationFunctionType.Silu  # test_sim overrides (CoreSim lacks Silu)


def _ensure_ntff_hook():
    """If the environment requests tracing (BASS_TRACE) but lacks the
    antenv.axon_hooks module, inject it so profiling degrades gracefully
    into working, instead of being skipped."""
    try:
        from antenv.axon_hooks import get_axon_ntff_profile_hook  # noqa: F401
        return
    except ImportError:
        pass
    try:
        import sys, types
        import antenv
        from trn_agent_boot.trn_boot import _ntff_profile_via_ctypes
        mod = types.ModuleType("antenv.axon_hooks")
        mod._hook = _ntff_profile_via_ctypes("/opt/axon/libaxon_pjrt.so")
        mod.set_axon_ntff_profile_hook = lambda h: setattr(mod, "_hook", h)
        mod.get_axon_ntff_profile_hook = lambda: mod._hook
        sys.modules["antenv.axon_hooks"] = mod
        antenv.axon_hooks = mod
    except Exception:
        pass


_BUILD_CACHE = {}
LAST_RESULT = None


def build(nt, tslice):
    """Build the SPMD program for NT token-tiles of 512 gathered tokens and
    a router slice of `tslice` tokens per core."""
    key = (nt, tslice)
    if key in _BUILD_CACHE:
        return _BUILD_CACHE[key]
    assert tslice % NTOK == 0
    ns_r = tslice // NTOK

    nc = bacc.Bacc("TRN2", target_bir_lowering=False, debug=False, num_devices=8)
    xtb = nc.dram_tensor("xtb", [nt, H, NTOK], f32r, kind="ExternalInput").ap()
    cwb = nc.dram_tensor("cwb", [nt, 128, NTOK], f32, kind="ExternalInput").ap()
    w1tb = nc.dram_tensor("w1tb", [2, KT_H, 128, IH], f32r, kind="ExternalInput").ap()
    w3tb = nc.dram_tensor("w3tb", [2, KT_H, 128, IH], f32r, kind="ExternalInput").ap()
    w2tb = nc.dram_tensor("w2tb", [2, KT_I, 128, H], f32r, kind="ExternalInput").ap()
    xrb = nc.dram_tensor("xrb", [ns_r, KT_H, 128, NTOK], f32r, kind="ExternalInput").ap()
    gwtb = nc.dram_tensor("gwtb", [KT_H, 128, E], f32r, kind="ExternalInput").ap()
    yb = nc.dram_tensor("yb", [2, nt, H, NTOK], f32, kind="ExternalOutput").ap()
    logb = nc.dram_tensor("logb", [E, tslice], f32, kind="ExternalOutput").ap()

    with tile.TileContext(nc) as tc:
        with (
            tc.tile_pool(name="wp", bufs=1) as wp,
            tc.tile_pool(name="xp", bufs=2) as xp,
            tc.tile_pool(name="pp", bufs=2) as pp,
            tc.tile_pool(name="cp", bufs=2) as cp,
            tc.tile_pool(name="yp", bufs=4) as yp,
            tc.tile_pool(name="rp", bufs=2) as rp,
            tc.tile_pool(name="gups", bufs=4, space="PSUM") as gups,
            tc.tile_pool(name="yps", bufs=2, space="PSUM") as yps,
            tc.tile_pool(name="lps", bufs=2, space="PSUM") as lps,
        ):
            # ---- router: logits slice [E, tslice] = gate_w @ x_slice.T ----
            gwt = rp.tile([128, KT_H * E], f32r, tag="gwt", bufs=1)
            for ki in range(KT_H):
                nc.sync.dma_start(out=gwt[:, ki * E:(ki + 1) * E], in_=gwtb[ki])
            for s in range(ns_r):
                xr = xp.tile([128, KT_H * NTOK], f32r, tag="x")
                for ki in range(KT_H):
                    nc.sync.dma_start(
                        out=xr[:, ki * NTOK:(ki + 1) * NTOK], in_=xrb[s, ki]
                    )
                lp = lps.tile([E, NTOK], f32, tag="lp")
                for ki in range(KT_H):
                    nc.tensor.matmul(
                        lp[:],
                        gwt[:, ki * E:(ki + 1) * E],
                        xr[:, ki * NTOK:(ki + 1) * NTOK],
                        start=(ki == 0),
                        stop=(ki == KT_H - 1),
                    )
                lsb = rp.tile([E, NTOK], f32, tag="lsb")
                nc.vector.tensor_copy(lsb[:], lp[:])
                nc.sync.dma_start(out=logb[:, s * NTOK:(s + 1) * NTOK], in_=lsb[:])

            # ---- expert FFN over gathered tokens, I in two halves ----
            for h in range(2):
                w1t = []
                w3t = []
                for ki in range(KT_H):
                    t1 = wp.tile([128, IH], f32r, tag=f"w1_{ki}", name=f"w1_{h}_{ki}")
                    nc.sync.dma_start(out=t1[:], in_=w1tb[h, ki])
                    w1t.append(t1)
                    t3 = wp.tile([128, IH], f32r, tag=f"w3_{ki}", name=f"w3_{h}_{ki}")
                    nc.sync.dma_start(out=t3[:], in_=w3tb[h, ki])
                    w3t.append(t3)
                w2t = []
                for ki in range(KT_I):
                    t2 = wp.tile([128, H], f32r, tag=f"w2_{ki}", name=f"w2_{h}_{ki}")
                    nc.sync.dma_start(out=t2[:], in_=w2tb[h, ki])
                    w2t.append(t2)

                for ti in range(nt):
                    xt = xp.tile([128, KT_H * NTOK], f32r, tag="x", name=f"x_{h}_{ti}")
                    for ki in range(KT_H):
                        nc.sync.dma_start(
                            out=xt[:, ki * NTOK:(ki + 1) * NTOK],
                            in_=xtb[ti].rearrange("(kt p) n -> kt p n", p=128)[ki],
                        )
                    cw = cp.tile([128, NTOK], f32, tag="cw", name=f"cw_{h}_{ti}")
                    nc.sync.dma_start(out=cw[:], in_=cwb[ti])

                    pt = pp.tile([128, ISUB * NTOK], f32r, tag="p", name=f"p_{h}_{ti}")
                    for isub in range(ISUB):
                        gp = gups.tile([128, NTOK], f32, tag="gu", name=f"g_{h}_{ti}_{isub}")
                        up = gups.tile([128, NTOK], f32, tag="gu", name=f"u_{h}_{ti}_{isub}")
                        msl = slice(isub * 128, (isub + 1) * 128)
                        for ki in range(KT_H):
                            nc.tensor.matmul(
                                gp[:],
                                w1t[ki][:, msl],
                                xt[:, ki * NTOK:(ki + 1) * NTOK],
                                start=(ki == 0),
                                stop=(ki == KT_H - 1),
                            )
                        for ki in range(KT_H):
                            nc.tensor.matmul(
                                up[:],
                                w3t[ki][:, msl],
                                xt[:, ki * NTOK:(ki + 1) * NTOK],
                                start=(ki == 0),
                                stop=(ki == KT_H - 1),
                            )
                        psl = slice(isub * NTOK, (isub + 1) * NTOK)
                        sg = yp.tile([128, NTOK], f32, tag="sg", name=f"sg_{h}_{ti}_{isub}")
                        nc.scalar.activation(sg[:], gp[:], ACT_FN)
                        # P = silu(G) * (U * cw); cw folded here (per-token col scale)
                        nc.vector.tensor_mul(pt[:, psl], up[:], cw[:])
                        nc.vector.tensor_mul(pt[:, psl], pt[:, psl], sg[:])
                    for m in range(KT_H):
                        yps_t = yps.tile([128, NTOK], f32, tag="y", name=f"y_{h}_{ti}_{m}")
                        for ki in range(KT_I):
                            nc.tensor.matmul(
                                yps_t[:],
                                w2t[ki][:, m * 128:(m + 1) * 128],
                                pt[:, ki * NTOK:(ki + 1) * NTOK],
                                start=(ki == 0),
                                stop=(ki == KT_I - 1),
                            )
                        ysb = yp.tile([128, NTOK], f32, tag="ysb", name=f"ysb_{h}_{ti}_{m}")
                        nc.vector.tensor_copy(ysb[:], yps_t[:])
                        nc.sync.dma_start(
                            out=yb[h, ti, m * 128:(m + 1) * 128, :], in_=ysb[:]
                        )
    nc.compile()
    _BUILD_CACHE[key] = nc
    return nc


def kernel(hidden_states, gate_w, e_bias, w1, w2, w3):
    _ensure_ntff_hook()
    B, S, Hd = hidden_states.shape
    assert Hd == H
    x = np.ascontiguousarray(hidden_states.reshape(-1, H).astype(np.float32))
    T = x.shape[0]
    tslice = T // 8

    # ---- host routing (float64 for stable top-k decisions) ----
    logits64 = x.astype(np.float64) @ gate_w.T.astype(np.float64)
    scores = 1.0 / (1.0 + np.exp(-logits64))
    biased = scores + e_bias.astype(np.float64)
    sel = np.argsort(-biased, axis=1, kind="stable")[:, :TOP_K]       # [T, 2]
    rw = np.take_along_axis(scores, sel, axis=1)                       # [T, 2]
    rw = rw / np.maximum(rw.sum(-1, keepdims=True), 1e-12)
    rw = rw.astype(np.float32)

    idx = []
    wts = []
    for e in range(E):
        mask = sel == e                                                # [T, 2]
        ide = np.where(mask.any(1))[0]
        pos = mask[ide].argmax(1)
        idx.append(ide)
        wts.append(rw[ide, pos])

    nmax = max(1, max(len(i) for i in idx))
    nt = (nmax + NTOK - 1) // NTOK
    R = nt * NTOK

    nc = build(nt, tslice)

    # ---- per-core inputs ----
    w1T = np.ascontiguousarray(np.transpose(w1, (0, 2, 1)).astype(np.float32))  # [E, H, I]
    w3T = np.ascontiguousarray(np.transpose(w3, (0, 2, 1)).astype(np.float32))  # [E, H, I]
    w2T = np.ascontiguousarray(np.transpose(w2, (0, 2, 1)).astype(np.float32))  # [E, I, H]
    gwT = np.ascontiguousarray(gate_w.T.astype(np.float32))                     # [H, E]
    gwtb = gwT.reshape(KT_H, 128, E)

    in_maps = []
    for c in range(E):
        n_c = len(idx[c])
        xg = np.zeros((R, H), np.float32)
        xg[:n_c] = x[idx[c]]
        xtb = np.ascontiguousarray(xg.reshape(nt, NTOK, H).transpose(0, 2, 1))
        cwf = np.zeros((R,), np.float32)
        cwf[:n_c] = wts[c]
        cwb = np.ascontiguousarray(
            np.broadcast_to(cwf.reshape(nt, 1, NTOK), (nt, 128, NTOK))
        )
        w1tb = np.ascontiguousarray(
            w1T[c].reshape(KT_H, 128, 2, IH).transpose(2, 0, 1, 3)
        )  # [2, KT_H, 128, IH]
        w3tb = np.ascontiguousarray(
            w3T[c].reshape(KT_H, 128, 2, IH).transpose(2, 0, 1, 3)
        )
        w2tb = np.ascontiguousarray(w2T[c].reshape(2, KT_I, 128, H))
        xsl = x[c * tslice:(c + 1) * tslice]                            # [tslice, H]
        xrb = np.ascontiguousarray(
            xsl.reshape(tslice // NTOK, NTOK, H).transpose(0, 2, 1)
            .reshape(tslice // NTOK, KT_H, 128, NTOK)
        )
        in_maps.append(
            {
                "xtb": xtb,
                "cwb": cwb,
                "w1tb": w1tb,
                "w3tb": w3tb,
                "w2tb": w2tb,
                "xrb": xrb,
                "gwtb": gwtb,
            }
        )

    res = run_bass_kernel_spmd(nc, in_maps, list(range(8)))
    global LAST_RESULT
    LAST_RESULT = res

    # ---- combine on host ----
    out = np.zeros((T, H), np.float32)
    logits_out = np.empty((T, E), np.float32)
    for c in range(E):
        r = res.results[c]
        y = r["yb"][0] + r["yb"][1]                                    # [nt, H, NTOK]
        y = y.transpose(0, 2, 1).reshape(R, H)[: len(idx[c])]
        out[idx[c]] += y
        logits_out[c * tslice:(c + 1) * tslice] = r["logb"].T
    return out.reshape(B, S, H), logits_out
